# revision 2
# baseline (speedup 1.0000x reference)
"""Trainium2 Bass kernel for a GNN message-passing layer.

Reference computation (all fp32):
    messages = h[src] @ W_msg.T            # [E, D]
    agg      = segment_sum(messages, dst)  # [N, D]
    out      = relu(concat(h, agg) @ W_upd.T + b_upd)

Algebraic restructure: segment_sum is linear, so
    agg = A @ W_msg.T          where A = segment_sum(h[src], dst)
and with W_upd = [Wu1 | Wu2], Wc = Wu2 @ W_msg (host-precomputed):
    out.T = relu(Wu1 @ h.T + Wc @ A.T + b)
so the device only computes A (gather h[src] + scatter-add by dst) plus
two small fused matmuls per 512-node region.

Design (v2 — replaces the ScalarE Sign-staircase kernel):
  - h is gathered in fp16 (256B rows; dma_gather int16 indices address
    even/odd strided views of h so src>>1 fits int16).
  - Per core, dst-sorted edges are cut into 128-edge chunks; each
    chunk's scatter one-hot [128, W] (W=32) is PRECOMPUTED ON HOST and
    streamed in one small DMA per region — no per-chunk ScalarE work.
  - Chunks accumulate into a per-512-node-region PSUM bank: a zeroing
    matmul (start=True) writes 0 across the bank, then every chunk
    matmul runs start=False at a fixed column-ladder offset; PSUM
    write-through/accumulate semantics handle chunk-boundary straddles.
  - The column ladder makes chunk PSUM offsets compile-time constants
    shared by all 8 SPMD cores: chunk j of a region serves dst columns
    < 8*(j+1), writing window [max(0, 8*(j+1)-W), 8*(j+1)); a host-side
    FIFO carry-queue assigns edges to chunks and asserts the carry
    lookback never exceeds W. Chunk counts per (region, parity) are the
    max over cores (compile-time list, ~3% padding).
  - All matmul operands fp16 (1 PE cycle/col vs 4 for fp32);
    rel_l2 error ~3.5e-4.

Measured bottleneck (wall-clock For_i-loop slope on HW): the SWDGE
gather descriptor path at ~3.2 ns/descriptor aggregate — one
descriptor per edge, Q7-ucode descriptor-generation-bound. Per-queue
gen is ~9.2 ns/desc; the (2-gathers-per-region, rotating queue pairs,
4-deep tile pool) layout below sustains ~2.8 queues. Splitting gathers
4-ways per region or onto one queue measured strictly worse. Compute
(PE/ScalarE/phase 2) fully overlaps under the gather: gather-only
variant == full kernel time. HW exec ~660 us vs ~724 us for the
staircase baseline.

Sharding: nodes partitioned contiguously across 8 cores by dst; each
core processes exactly the edges targeting its node shard; no
collectives. Output is out.T per core, re-assembled on host.
"""

import contextlib

import numpy as np

import concourse.mybir as mybir
import concourse.tile as tile
from concourse import bacc
from concourse.bass_utils import run_bass_kernel_spmd

P = 128  # SBUF partitions
D = 128  # feature dim (in_dim == out_dim == 128)
N_CORES = 8
CHUNK = 128  # edges per matmul chunk
RW = 512  # region width in node columns (one PSUM bank)
LAD = 8  # ladder stride: chunk j serves dst columns < LAD*(j+1)
GAT_BUFS = 4  # gather tile pool depth

_prog_cache: dict = {}


def _build_program(N2, SP, W, layout, loop_iters=None):
    """One SPMD program shared by all 8 cores.

    N2     : rows of the fp16 h table incl. 2 appended zero rows
    SP     : padded nodes per core
    W      : one-hot window width (ladder step + carry lookback)
    layout : tuple over regions of (KE, KO) chunk counts
    loop_iters : if set, wrap the compute body in a For_i hardware loop
                 (wall-clock slope timing harness)
    """
    f16 = mybir.dt.float16
    f32 = mybir.dt.float32
    i16 = mybir.dt.int16

    NR = len(layout)
    total_chunks = sum(ke + ko for ke, ko in layout)
    IW = total_chunks * 8  # idx columns (16-wrapped, x8 replicated)

    nc = bacc.Bacc("TRN2", target_bir_lowering=False, num_swdge_queues=4)

    h_d = nc.dram_tensor("h", [N2, D], f16, kind="ExternalInput")
    hsT_d = nc.dram_tensor("hsT", [P, SP], f16, kind="ExternalInput")
    idx_d = nc.dram_tensor("idx", [P, IW], i16, kind="ExternalInput")
    rhs_d = nc.dram_tensor("rhs", [P, total_chunks * W], f16, kind="ExternalInput")
    w1_d = nc.dram_tensor("w1T", [D, D], f16, kind="ExternalInput")
    wc_d = nc.dram_tensor("wcT", [D, D], f16, kind="ExternalInput")
    b_d = nc.dram_tensor("bias", [P, 1], f32, kind="ExternalInput")
    z_d = nc.dram_tensor("zeros", [P, RW], f16, kind="ExternalInput")
    out_d = nc.dram_tensor("outT", [P, SP], f32, kind="ExternalOutput")

    h_even = h_d[0:N2:2, :]
    h_odd = h_d[1:N2:2, :]

    with tile.TileContext(nc) as tc:
        with (
            tc.tile_pool(name="constp", bufs=1) as constp,
            tc.tile_pool(name="gatp", bufs=GAT_BUFS) as gatp,
            tc.tile_pool(name="rhsp", bufs=3) as rhsp,
            tc.tile_pool(name="atp", bufs=3) as atp,
            tc.tile_pool(name="outp", bufs=3) as outp,
            tc.tile_pool(name="psp", bufs=4, space="PSUM") as psp,
            tc.tile_pool(name="ps2p", bufs=2, space="PSUM") as ps2p,
        ):
            w1_t = constp.tile([D, D], f16)
            nc.sync.dma_start(w1_t[:], w1_d[:])
            wc_t = constp.tile([D, D], f16)
            nc.sync.dma_start(wc_t[:], wc_d[:])
            b_t = constp.tile([P, 1], f32)
            nc.sync.dma_start(b_t[:], b_d[:])
            z_t = constp.tile([P, RW], f16)
            nc.sync.dma_start(z_t[:], z_d[:])
            idx_t = constp.tile([P, IW], i16)
            nc.sync.dma_start(idx_t[:], idx_d[:])
            hsT_t = constp.tile([P, SP], f16)
            nc.sync.dma_start(hsT_t[:], hsT_d[:])

            loop_cm = (
                tc.For_i(0, loop_iters, 1)
                if loop_iters is not None
                else contextlib.nullcontext()
            )
            with loop_cm:
                icol = 0  # running idx column offset
                jch = 0  # running global chunk index
                for r in range(NR):
                    KE, KO = layout[r]
                    NCH = KE + KO
                    rw = min(RW, SP - r * RW)

                    g_t = gatp.tile([P, max(NCH, 1) * D], f16)
                    if NCH > 0:
                        g3 = g_t[:].rearrange("p (c d) -> p c d", c=NCH)
                        subs = []
                        if KE > 0:
                            subs.append((0, KE, h_even, (2 * r) % 4))
                        if KO > 0:
                            subs.append((KE, NCH, h_odd, (2 * r + 1) % 4))
                        for c0, c1, h_src, q in subs:
                            nk = c1 - c0
                            nc.gpsimd.dma_gather(
                                out_ap=g3[:, c0:c1, :],
                                in_ap=h_src,
                                idxs_ap=idx_t[:, icol + c0 * 8 : icol + c1 * 8],
                                num_idxs=nk * CHUNK,
                                num_idxs_reg=nk * CHUNK,
                                elem_size=D,
                                elem_step=2 * D,
                                single_packet=False,
                                queue_num=q,
                            )
                        icol += NCH * 8

                    rhs_t = rhsp.tile([P, max(NCH, 1) * W], f16)
                    if NCH > 0:
                        nc.sync.dma_start(
                            rhs_t[:], rhs_d[:, jch * W : (jch + NCH) * W]
                        )

                    # phase 1: zero the region bank, then accumulate chunks
                    ps_t = psp.tile([P, RW], f32)
                    nc.tensor.matmul(
                        out=ps_t[:, :rw],
                        lhsT=z_t[:, 0:D],
                        rhs=z_t[:, 0:rw],
                        start=True,
                        stop=(NCH == 0),
                    )
                    for par, kpar in ((0, KE), (1, KO)):
                        for c in range(kpar):
                            cc = c if par == 0 else KE + c
                            off = max(0, min(LAD * (c + 1), rw) - W)
                            nc.tensor.matmul(
                                out=ps_t[:, off : off + W],
                                lhsT=g_t[:, cc * D : (cc + 1) * D],
                                rhs=rhs_t[:, cc * W : (cc + 1) * W],
                                start=False,
                                stop=(cc == NCH - 1),
                            )
                    jch += NCH

                    # A.T region -> SBUF (fp16)
                    at_t = atp.tile([P, RW], f16)
                    nc.scalar.activation(
                        out=at_t[:, :rw],
                        in_=ps_t[:, :rw],
                        func=mybir.ActivationFunctionType.Copy,
                    )

                    # phase 2: out.T = relu(Wu1 @ h.T + Wc @ A.T + b)
                    col = r * RW
                    ps2_t = ps2p.tile([P, RW], f32)
                    nc.tensor.matmul(
                        out=ps2_t[:, :rw],
                        lhsT=w1_t[:],
                        rhs=hsT_t[:, col : col + rw],
                        start=True,
                        stop=False,
                    )
                    nc.tensor.matmul(
                        out=ps2_t[:, :rw],
                        lhsT=wc_t[:],
                        rhs=at_t[:, :rw],
                        start=False,
                        stop=True,
                    )
                    o_t = outp.tile([P, RW], f32)
                    nc.scalar.activation(
                        o_t[:, :rw],
                        ps2_t[:, :rw],
                        mybir.ActivationFunctionType.Relu,
                        bias=b_t[:],
                    )
                    nc.sync.dma_start(out_d[:, col : col + rw], o_t[:, :rw])

    nc.compile()
    return nc


def _pack_core(dl, sl, NR, SP):
    """Minimum chunk counts for one core under the ladder (FIFO sim).

    dl : dst column of each edge, sorted ascending
    sl : src node of each edge (same order)
    """
    packs = {}
    for r in range(NR):
        rw = min(RW, SP - r * RW)
        lo = np.searchsorted(dl, r * RW)
        hi = np.searchsorted(dl, r * RW + rw)
        for par in (0, 1):
            m = (sl[lo:hi] & 1) == par
            cols = dl[lo:hi][m] - r * RW  # ascending
            srcs = sl[lo:hi][m]
            n = len(cols)
            need = 0
            i = 0
            while i < n:
                limit = min(LAD * (need + 1), rw)
                if limit >= rw:
                    take = min(CHUNK, n - i)
                else:
                    take = min(CHUNK, np.searchsorted(cols, limit) - i)
                i += take
                need += 1
            packs[(r, par)] = (need, cols, srcs)
    return packs


def _prep_inputs(h, edge_index, W_msg, W_upd, b_upd):
    """Host-side sharding: dst-sort edges per core, pack 128-edge chunks
    under the column ladder, build int16 gather indices and fp16 one-hot
    scatter matrices."""
    N0, d = h.shape
    assert d == D

    SP = -(-N0 // (N_CORES * P)) * P  # padded nodes per core
    NR = -(-SP // RW)
    W = 4 * LAD  # one-hot window; asserts below guard the carry lookback

    src = np.ascontiguousarray(edge_index[0]).astype(np.int64)
    dst = np.ascontiguousarray(edge_index[1]).astype(np.int64)

    N2 = N0 + 2
    hg = np.zeros((N2, D), dtype=np.float16)
    hg[:N0] = h.astype(np.float16)
    pad_idx = N0 // 2  # row N0 (even) / N0+1 (odd), both zero

    order = np.argsort(dst, kind="stable")
    src_s = src[order]
    dst_s = dst[order]
    core_start = np.searchsorted(dst_s, np.arange(N_CORES + 1) * SP)

    # pass 1: per-core packing, global chunk-count layout (max over cores)
    all_packs = []
    for c in range(N_CORES):
        lo, hi = core_start[c], core_start[c + 1]
        all_packs.append(_pack_core(dst_s[lo:hi] - c * SP, src_s[lo:hi], NR, SP))
    layout = tuple(
        (
            max(p[(r, 0)][0] for p in all_packs),
            max(p[(r, 1)][0] for p in all_packs),
        )
        for r in range(NR)
    )
    total_chunks = sum(ke + ko for ke, ko in layout)

    # pass 2: build per-core streams
    w1T = np.ascontiguousarray(W_upd[:, :D].T.astype(np.float16))
    wc = (W_upd[:, D:].astype(np.float64) @ W_msg.astype(np.float64)).astype(
        np.float32
    )
    wcT = np.ascontiguousarray(wc.T.astype(np.float16))
    bias = np.ascontiguousarray(b_upd.astype(np.float32).reshape(P, 1))
    zeros = np.zeros((P, RW), dtype=np.float16)

    in_maps = []
    for c in range(N_CORES):
        packs = all_packs[c]
        idx_cols = np.empty((16, total_chunks * 8), dtype=np.int16)
        rhs = np.zeros((P, total_chunks * W), dtype=np.float16)
        jch = 0
        for r in range(NR):
            rw = min(RW, SP - r * RW)
            KE, KO = layout[r]
            for par, kpar in ((0, KE), (1, KO)):
                _, cols, srcs = packs[(r, par)]
                n = len(cols)
                cap = kpar * CHUNK
                idx_flat = np.full(cap, pad_idx, dtype=np.int16)
                i = 0
                for j in range(kpar):
                    limit = min(LAD * (j + 1), rw)
                    off = max(0, limit - W)
                    if limit >= rw:
                        take = min(CHUNK, n - i)
                    else:
                        take = min(CHUNK, int(np.searchsorted(cols, limit)) - i)
                    if take > 0:
                        ccols = cols[i : i + take] - off
                        assert ccols.min() >= 0, "carry lookback exceeded W"
                        assert ccols.max() < W
                        sslot = np.arange(take)
                        idx_flat[j * CHUNK + sslot] = (
                            srcs[i : i + take] >> 1
                        ).astype(np.int16)
                        rhs[sslot, (jch + j) * W + ccols] = 1.0
                        i += take
                assert i == n, f"unconsumed edges core={c} r={r} par={par}"
                # idx wrap [cap] -> [16, cap/16] (replicated x8 below)
                seg = idx_flat.reshape(cap // 16, 16).T
                idx_cols[:, jch * 8 : jch * 8 + cap // 16] = seg
                jch += kpar
        idx_in = np.tile(idx_cols, (8, 1))

        lo_n = c * SP
        hi_n = min((c + 1) * SP, N0)
        hs = np.zeros((SP, D), dtype=np.float32)
        if hi_n > lo_n:
            hs[: hi_n - lo_n] = h[lo_n:hi_n]

        in_maps.append(
            {
                "h": hg,
                "hsT": np.ascontiguousarray(hs.T.astype(np.float16)),
                "idx": np.ascontiguousarray(idx_in),
                "rhs": rhs,
                "w1T": w1T,
                "wcT": wcT,
                "bias": bias,
                "zeros": zeros,
            }
        )
    return in_maps, N2, SP, W, layout


def kernel_with_results(
    h, edge_index, W_msg, W_upd, b_upd, loop_iters=None, **run_kwargs
):
    in_maps, N2, SP, W, layout = _prep_inputs(h, edge_index, W_msg, W_upd, b_upd)

    key = (N2, SP, W, layout, loop_iters)
    if key not in _prog_cache:
        _prog_cache[key] = _build_program(N2, SP, W, layout, loop_iters=loop_iters)
    nc = _prog_cache[key]

    res = run_bass_kernel_spmd(
        nc, in_maps, core_ids=list(range(N_CORES)), **run_kwargs
    )

    N0 = N2 - 2
    out = np.empty((N0, D), dtype=np.float32)
    for c in range(N_CORES):
        lo = c * SP
        hi = min((c + 1) * SP, N0)
        if hi > lo:
            out[lo:hi] = res.results[c]["outT"].T[: hi - lo]
    return out, res


def kernel(h, edge_index, W_msg, W_upd, b_upd):
    out, _ = kernel_with_results(h, edge_index, W_msg, W_upd, b_upd)
    return out
